# revision 1
# baseline (speedup 1.0000x reference)
"""ColorRandomizer Trainium2 kernel: brightness/contrast/saturation/hue on 8 cores.

Data-parallel: 4 images per core. Per image:
  ph1  x1 = min(x*bf, 1)                (DVE tensor_scalar, fp32->fp16, + free accum for mean)
  ph2  x2 = clip01(cf*x1 + (1-cf)*mean) (ACT relu affine + DVE min)
  ph3  x3 = clip01(sf*x2 + (1-sf)*gray(x2))
  ph4  HSV hue shift, reformulated:
         J = cr*H6 built w/o division via case-select (copy_predicated),
         i6 = J * exp(-ln(cr+eps)) + 6*hf,
         out_c = min(minc + cr*relu(min(|i6+a_c|,|i6+a_c-6|) - 1), maxc)
       (tent identity for HSV->RGB; no floor/mod needed for |hf|<=1/3)
Storage fp16 (validated absmax ~6.5e-3 vs fp32 reference), scalars/accums fp32.
"""
import sys

for _p in ("/opt/trn_rl_repo",):
    if _p not in sys.path:
        sys.path.append(_p)

import numpy as np
from concourse import bass, bacc, mybir, tile, bass_isa
from concourse.bass_utils import run_bass_kernel_spmd

F32 = mybir.dt.float32
F16 = mybir.dt.float16
OP = mybir.AluOpType
AF = mybir.ActivationFunctionType

NIMG = 4          # images per core
H, W = 480, 640
NPIX = H * W      # 307200
F = NPIX // 128   # 2400 free elems per partition per plane
F3 = 3 * F
GRAY_W = (0.299, 0.587, 0.114)

TRACE = False     # test.py flips this for profiling
_CACHE = {}


def _build():
    nc = bacc.Bacc(None, target_bir_lowering=False)
    x_h = nc.declare_dram_parameter("x", [NIMG, 3, H, W], F32, isOutput=False)
    fac_h = nc.declare_dram_parameter("fac", [NIMG, 8], F32, isOutput=False)
    y_h = nc.declare_dram_parameter("y", [NIMG, 3, H, W], F32, isOutput=True)

    dma = nc.sync  # HWDGE

    # activation float biases must exist as const APs
    for v in (1e-30, -1.0, 3.0, -3.0, -5.0, -7.0):
        t = nc.alloc_sbuf_tensor(f"cst-{v}", [128, 1], F32)
        nc.gpsimd.memset(t.ap(), v)
        nc.const_aps.aps[(F32, v)] = t.ap()
    nc.all_engine_barrier()

    with tile.TileContext(nc) as tc:
        with tc.tile_pool(name="p", bufs=1) as pool:
            # broadcast per-image factors to all partitions once
            fac1 = pool.tile([1, NIMG * 8], F32)
            dma.dma_start(fac1[:], fac_h[:].flatten()[None, :])
            facb = pool.tile([128, NIMG * 8], F32)
            nc.gpsimd.partition_broadcast(facb[:], fac1[:], channels=128)

            def col(i, k):
                return facb[:, i * 8 + k : i * 8 + k + 1]

            for i in range(NIMG):
                # ---- load ----
                xin = pool.tile([128, F3], F32, tag="io", bufs=2)
                for c in range(3):
                    dma.dma_start(
                        xin[:, c * F : (c + 1) * F],
                        x_h[i, c].flatten().rearrange("(p f) -> p f", p=128),
                    )

                # ---- ph1: brightness + per-channel sums ----
                rgb = pool.tile([128, F3], F16, tag="rgb", bufs=2)
                sums = pool.tile([128, 4], F32, tag="sums", bufs=2)
                jt = pool.tile([128, F3], F16, tag="jt")
                for c in range(3):
                    nc.vector.tensor_scalar(
                        rgb[:, c * F : (c + 1) * F],
                        xin[:, c * F : (c + 1) * F],
                        col(i, 0), 1.0, OP.mult, OP.min,
                    )
                    # per-channel sums for the contrast mean (ACT accum; DVE
                    # tensor_scalar accum_out is broken on HW)
                    nc.scalar.activation(
                        jt[:, c * F : (c + 1) * F],
                        rgb[:, c * F : (c + 1) * F],
                        AF.Identity, bias=0.0, scale=1.0,
                        accum_out=sums[:, c : c + 1],
                    )
                # weighted per-partition sum -> all-core scalar -> delta
                ws = pool.tile([128, 1], F32, tag="ws", bufs=2)
                nc.vector.tensor_scalar(ws[:], sums[:, 0:1], GRAY_W[0], None, OP.mult)
                ws2 = pool.tile([128, 1], F32, tag="ws2", bufs=2)
                nc.vector.scalar_tensor_tensor(ws2[:], sums[:, 1:2], GRAY_W[1], ws[:], OP.mult, OP.add)
                ws3 = pool.tile([128, 1], F32, tag="ws3", bufs=2)
                nc.vector.scalar_tensor_tensor(ws3[:], sums[:, 2:3], GRAY_W[2], ws2[:], OP.mult, OP.add)
                ssum = pool.tile([128, 1], F32, tag="ssum", bufs=2)
                nc.gpsimd.partition_all_reduce(ssum[:], ws3[:], 128, bass_isa.ReduceOp.add)
                delta = pool.tile([128, 1], F32, tag="delta", bufs=2)
                nc.vector.tensor_tensor(delta[:], ssum[:], col(i, 2), OP.mult)

                # ---- ph2: contrast (upper clip fused into ph3 consumers) ----
                ya = pool.tile([128, F3], F16, tag="ya", bufs=2)
                nc.scalar.activation(rgb[:], rgb[:], AF.Relu, bias=delta[:], scale=col(i, 1))

                # ---- ph3: saturation;  gs = (1-sf)*gray(x2) built in ya ----
                # each consumer applies the pending "min 1" via fused 2-scalar TS
                nc.vector.tensor_scalar(ya[:, 0:F], rgb[:, 0:F], 1.0, col(i, 4), OP.min, OP.mult)
                nc.vector.tensor_scalar(ya[:, F:2 * F], rgb[:, F:2 * F], 1.0, col(i, 5), OP.min, OP.mult)
                nc.vector.tensor_tensor(ya[:, 2 * F:3 * F], ya[:, F:2 * F], ya[:, 0:F], OP.add)
                nc.vector.tensor_scalar(ya[:, 0:F], rgb[:, 2 * F:3 * F], 1.0, col(i, 6), OP.min, OP.mult)
                nc.vector.tensor_tensor(ya[:, F:2 * F], ya[:, 0:F], ya[:, 2 * F:3 * F], OP.add)
                y3 = pool.tile([128, F3], F16, tag="y3")
                nc.vector.tensor_scalar(y3[:], rgb[:], 1.0, col(i, 3), OP.min, OP.mult)
                gsb = ya[:, F:2 * F][:, None, :].broadcast_to([128, 3, F])
                nc.vector.tensor_tensor(
                    jt[:].rearrange("p (c f) -> p c f", c=3),
                    y3[:].rearrange("p (c f) -> p c f", c=3),
                    gsb, OP.add,
                )
                nc.vector.tensor_scalar(rgb[:], jt[:], 0.0, 1.0, OP.max, OP.min)

                # ---- ph4: hue ----
                # ya: [0:F]=maxc  [F:2F]=minc  [2F:3F]=cr   (2F used as scratch first)
                nc.vector.tensor_tensor(ya[:, 2 * F:3 * F], rgb[:, 0:F], rgb[:, F:2 * F], OP.max)
                nc.vector.tensor_tensor(ya[:, 0:F], ya[:, 2 * F:3 * F], rgb[:, 2 * F:3 * F], OP.max)
                nc.vector.tensor_tensor(ya[:, 2 * F:3 * F], rgb[:, 0:F], rgb[:, F:2 * F], OP.min)
                nc.vector.tensor_tensor(ya[:, F:2 * F], ya[:, 2 * F:3 * F], rgb[:, 2 * F:3 * F], OP.min)
                nc.vector.tensor_tensor(ya[:, 2 * F:3 * F], ya[:, 0:F], ya[:, F:2 * F], OP.subtract)
                # masks: y3[0:F]=(r>=maxc) y3[F:2F]=(g>=maxc); d1 -> y3[2F:3F]
                mxb = ya[:, 0:F][:, None, :].broadcast_to([128, 2, F])
                nc.vector.tensor_tensor(
                    y3[:, 0:2 * F].bitcast(mybir.dt.int16).rearrange("p (c f) -> p c f", c=2),
                    rgb[:, 0:2 * F].rearrange("p (c f) -> p c f", c=2),
                    mxb, OP.is_ge,
                )
                nc.vector.tensor_tensor(y3[:, 2 * F:3 * F], rgb[:, F:2 * F], rgb[:, 2 * F:3 * F], OP.subtract)
                # jg = 2cr + (b - r)
                nc.vector.tensor_tensor(jt[:, 0:F], rgb[:, 2 * F:3 * F], rgb[:, 0:F], OP.subtract)
                nc.vector.tensor_scalar(jt[:, 2 * F:3 * F], ya[:, 2 * F:3 * F], 2.0, None, OP.mult)
                nc.vector.tensor_tensor(jt[:, F:2 * F], jt[:, 2 * F:3 * F], jt[:, 0:F], OP.add)
                # J = 4cr + (r - g), then case overrides
                nc.vector.tensor_tensor(jt[:, 0:F], rgb[:, 0:F], rgb[:, F:2 * F], OP.subtract)
                nc.vector.tensor_scalar(jt[:, 2 * F:3 * F], ya[:, 2 * F:3 * F], 4.0, None, OP.mult)
                Jt = pool.tile([128, F], F16, tag="Jt")
                nc.vector.tensor_tensor(Jt[:], jt[:, 2 * F:3 * F], jt[:, 0:F], OP.add)
                nc.vector.copy_predicated(Jt[:], y3[:, F:2 * F].bitcast(mybir.dt.int16), jt[:, F:2 * F])
                nc.vector.copy_predicated(Jt[:], y3[:, 0:F].bitcast(mybir.dt.int16), y3[:, 2 * F:3 * F])
                # invc = exp(-ln(cr+eps)) on ACT (f32)
                lc = pool.tile([128, F], F32, tag="lc")
                nc.scalar.activation(lc[:], ya[:, 2 * F:3 * F], AF.Ln, bias=1e-30)
                nc.scalar.activation(lc[:], lc[:], AF.Exp, scale=-1.0)
                # i6 = J*invc + 6hf
                nc.vector.tensor_tensor(jt[:, F:2 * F], Jt[:], lc[:], OP.mult)
                nc.vector.tensor_scalar(jt[:, 0:F], jt[:, F:2 * F], col(i, 7), None, OP.add)
                # recon: A1=|i6+a|, A2=|i6+a-6| per channel (ACT)
                A1 = pool.tile([128, F3], F16, tag="A1")
                A2 = pool.tile([128, F3], F16, tag="A2")
                for ci, a in enumerate((3.0, 1.0, -1.0)):
                    nc.scalar.activation(A1[:, ci * F:(ci + 1) * F], jt[:, 0:F], AF.Abs, bias=a)
                    nc.scalar.activation(A2[:, ci * F:(ci + 1) * F], jt[:, 0:F], AF.Abs, bias=a - 6.0)
                nc.vector.tensor_tensor(y3[:], A1[:], A2[:], OP.min)
                nc.scalar.activation(A1[:], y3[:], AF.Relu, bias=-1.0)
                crb = ya[:, 2 * F:3 * F][:, None, :].broadcast_to([128, 3, F])
                mnb = ya[:, F:2 * F][:, None, :].broadcast_to([128, 3, F])
                mxb3 = ya[:, 0:F][:, None, :].broadcast_to([128, 3, F])
                v3 = lambda t: t[:].rearrange("p (c f) -> p c f", c=3)
                nc.vector.tensor_tensor(v3(A2), v3(A1), crb, OP.mult)
                nc.vector.tensor_tensor(v3(A1), v3(A2), mnb, OP.add)
                nc.vector.tensor_tensor(v3(A2), v3(A1), mxb3, OP.min)
                o3 = pool.tile([128, F3], F32, tag="io", bufs=2)
                nc.scalar.activation(o3[:], A2[:], AF.Copy)

                # ---- store ----
                for c in range(3):
                    dma.dma_start(
                        y_h[i, c].flatten().rearrange("(p f) -> p f", p=128),
                        o3[:, c * F : (c + 1) * F],
                    )

    nc.finalize()
    return nc


def _get_nc():
    if "nc" not in _CACHE:
        _CACHE["nc"] = _build()
    return _CACHE["nc"]


def kernel(x, brightness_f, contrast_f, saturation_f, hue_f, num_samples=1, **_):
    x = np.ascontiguousarray(np.asarray(x, dtype=np.float32))
    bf = np.asarray(brightness_f, np.float32)
    cf = np.asarray(contrast_f, np.float32)
    sf = np.asarray(saturation_f, np.float32)
    hf = np.asarray(hue_f, np.float32)
    B = x.shape[0]
    fac = np.stack(
        [
            bf, cf, (1.0 - cf) / np.float32(NPIX), sf,
            GRAY_W[0] * (1.0 - sf), GRAY_W[1] * (1.0 - sf), GRAY_W[2] * (1.0 - sf),
            6.0 * hf,
        ],
        axis=1,
    ).astype(np.float32)

    nc = _get_nc()
    in_maps = [
        {"x": x[k * NIMG:(k + 1) * NIMG], "fac": fac[k * NIMG:(k + 1) * NIMG]}
        for k in range(8)
    ]
    res = run_bass_kernel_spmd(nc, in_maps, core_ids=list(range(8)), trace=TRACE)
    if TRACE:
        _CACHE["last"] = res
    out = np.concatenate([res.results[k]["y"] for k in range(8)], axis=0)
    return out.astype(np.float32)



# revision 10
# speedup vs baseline: 1.4252x; 1.4252x over previous
"""ColorRandomizer Trainium2 kernel: brightness/contrast/saturation/hue on 8 cores.

Data-parallel: 4 images per core, f16 I/O (host converts), 2 chunks per image.

Per image (factors b,c,s,h; W = gray weights):
  A:  x1 = min(b*x, 1); per-channel sums -> m -> d=(1-c)m, e'=min(c+d,1)
  B:  u = relu(c*x1 + d)                       (ACT)
  C:  g = sum_c (1-s)W_c*min(u_c,e');  x3 = clip01(s*min(u,e') + g)
  D:  hue via tent identity, division-free selection:
      p1=(g3-b3)/cr, p2=(b3-r3)/cr, i6 = select(p1 | 2+p2 | 4-p1-p2) by argmax
      (masks from x1; valid by hue continuity at clamp-induced ties)
  E:  out_c = minc + cr*(clip(| |i6+gam_c| - 3 |, 1, 2) - 1),  gam=(6h,6h-2,6h-4)
Engines: DVE TS(4x)/TT(2x) + ACT (sums/relu/ln/exp/abs) + GpSimd (max/min chains).
"""
import sys

for _p in ("/opt/trn_rl_repo",):
    if _p not in sys.path:
        sys.path.append(_p)

import numpy as np
from concourse import bass, bacc, mybir, tile, bass_isa
from concourse.bass_utils import run_bass_kernel_spmd

F32 = mybir.dt.float32
F16 = mybir.dt.float16
I16 = mybir.dt.int16
OP = mybir.AluOpType
AF = mybir.ActivationFunctionType

NIMG = 4          # images per core
H, W = 480, 640
NPIX = H * W      # 307200
F = NPIX // 128   # 2400 free elems per partition per channel plane
NCH = 2           # chunks per image
F2 = F // NCH     # 1200
GRAY_W = (0.299, 0.587, 0.114)
EPS = 1e-4
NFAC = 12

TRACE = False     # test.py flips this for profiling
_CACHE = {}


def _build():
    nc = bacc.Bacc(None, target_bir_lowering=False)
    x_h = nc.declare_dram_parameter("x", [NIMG, 3, H, W], F16, isOutput=False)
    fac_h = nc.declare_dram_parameter("fac", [NIMG, NFAC], F32, isOutput=False)
    y_h = nc.declare_dram_parameter("y", [NIMG, 3, H, W], F16, isOutput=True)

    dma = nc.sync  # HWDGE

    # activation float biases must exist as const APs
    for v in (EPS, -3.0):
        t = nc.alloc_sbuf_tensor(f"cst-{v}", [128, 1], F32)
        nc.gpsimd.memset(t.ap(), v)
        nc.const_aps.aps[(F32, v)] = t.ap()
    nc.all_engine_barrier()

    with tile.TileContext(nc) as tc:
        with tc.tile_pool(name="p", bufs=1) as pool:
            # broadcast per-image factors to all partitions once
            fac1 = pool.tile([1, NIMG * NFAC], F32)
            dma.dma_start(fac1[:], fac_h[:].flatten()[None, :])
            facb = pool.tile([128, NIMG * NFAC], F32)
            nc.gpsimd.partition_broadcast(facb[:], fac1[:], channels=128)

            def col(i, k):
                return facb[:, i * NFAC + k : i * NFAC + k + 1]

            x1s, ds, es, ses = {}, {}, {}, {}

            def phase1(i):
                # ---- load + brightness + sums ----
                xin = pool.tile([128, 3 * F], F16, tag="xin", bufs=2)
                xv = x_h[i].flatten().rearrange("(c p f) -> p c f", c=3, p=128)
                dma.dma_start(xin[:].rearrange("p (c f) -> p c f", c=3), xv)
                x1 = pool.tile([128, 3 * F], F16, tag="x1", bufs=3)
                nc.vector.tensor_scalar(x1[:], xin[:], col(i, 0), 1.0, OP.mult, OP.min)
                sums = pool.tile([128, 4], F32, tag="sums", bufs=2)
                for c in range(3):
                    dead = pool.tile([128, F], F16, tag="dead", bufs=1)
                    nc.scalar.activation(
                        dead[:], x1[:, c * F : (c + 1) * F],
                        AF.Identity, bias=0.0, scale=1.0,
                        accum_out=sums[:, c : c + 1],
                    )
                # d = sum_c kd_c*sums_c (kd = (1-c)W_c/NPIX), then all-reduce
                ws = pool.tile([128, 1], F32, tag="ws", bufs=2)
                nc.vector.tensor_scalar(ws[:], sums[:, 0:1], col(i, 9), None, OP.mult)
                ws2 = pool.tile([128, 1], F32, tag="ws2", bufs=2)
                nc.vector.scalar_tensor_tensor(ws2[:], sums[:, 1:2], col(i, 10), ws[:], OP.mult, OP.add)
                ws3 = pool.tile([128, 1], F32, tag="ws3", bufs=2)
                nc.vector.scalar_tensor_tensor(ws3[:], sums[:, 2:3], col(i, 11), ws2[:], OP.mult, OP.add)
                d_t = pool.tile([128, 1], F32, tag="d", bufs=3)
                nc.gpsimd.partition_all_reduce(d_t[:], ws3[:], 128, bass_isa.ReduceOp.add)
                e0 = pool.tile([128, 1], F32, tag="e0", bufs=3)
                nc.vector.scalar_tensor_tensor(e0[:], d_t[:], 1.0, facb[:, i * NFAC + 1 : i * NFAC + 2], OP.mult, OP.add)
                e_t = pool.tile([128, 1], F32, tag="e", bufs=3)
                nc.vector.tensor_scalar(e_t[:], e0[:], 1.0, None, OP.min)
                se_t = pool.tile([128, 1], F32, tag="se", bufs=3)
                nc.vector.tensor_tensor(se_t[:], e_t[:], col(i, 2), OP.mult)
                x1s[i], ds[i], es[i], ses[i] = x1, d_t, e_t, se_t

            def phase2(i, k):
                x1, d_t, e_t, se_t = x1s[i], ds[i], es[i], ses[i]
                lo = k * F2

                def xch(c):  # x1 channel-chunk view [128, F2]
                    return x1[:, c * F + lo : c * F + lo + F2]

                # B: u = relu(c*x1 + d)   (ACT, per channel)
                u = pool.tile([128, 3 * F2], F16, tag="u", bufs=2)
                x1v = x1[:, lo : lo + F2]  # [128, F2] base; 3ch strided view below
                nc.scalar.activation(
                    u[:].rearrange("p (c f) -> p c f", c=3),
                    x1[:].rearrange("p (c f) -> p c f", c=3)[:, :, lo : lo + F2],
                    AF.Relu, bias=d_t[:], scale=col(i, 1),
                )

                def uch(c):
                    return u[:, c * F2 : (c + 1) * F2]

                # C: gray-blend
                wx2 = pool.tile([128, 3 * F2], F16, tag="wx2", bufs=2)
                for c in range(3):
                    nc.vector.tensor_scalar(
                        wx2[:, c * F2 : (c + 1) * F2], uch(c),
                        e_t[:], col(i, 3 + c), OP.min, OP.mult,
                    )
                ga = pool.tile([128, F2], F16, tag="sm3", bufs=2)
                nc.vector.tensor_tensor(ga[:], wx2[:, 0:F2], wx2[:, F2:2 * F2], OP.add)
                g = pool.tile([128, F2], F16, tag="sm6", bufs=2)
                nc.vector.tensor_tensor(g[:], ga[:], wx2[:, 2 * F2:3 * F2], OP.add)
                sx2 = pool.tile([128, 3 * F2], F16, tag="sx2", bufs=2)
                nc.vector.tensor_scalar(sx2[:], u[:], col(i, 2), se_t[:], OP.mult, OP.min)
                y = pool.tile([128, 3 * F2], F16, tag="y", bufs=2)
                gb = g[:][:, None, :].broadcast_to([128, 3, F2])
                nc.vector.tensor_tensor(
                    y[:].rearrange("p (c f) -> p c f", c=3),
                    sx2[:].rearrange("p (c f) -> p c f", c=3),
                    gb, OP.add,
                )
                x3 = pool.tile([128, 3 * F2], F16, tag="x3", bufs=2)
                nc.vector.tensor_scalar(x3[:], y[:], 0.0, 1.0, OP.max, OP.min)

                def x3ch(c):
                    return x3[:, c * F2 : (c + 1) * F2]

                # D: maxc/minc (GpSimd), cr, diffs, reciprocal, i6 select
                U3 = pool.tile([128, F2], F16, tag="sm1", bufs=2)
                nc.vector.tensor_tensor(U3[:], x3ch(1), x3ch(2), OP.max)
                mxc = pool.tile([128, F2], F16, tag="sm7", bufs=2)
                nc.vector.tensor_tensor(mxc[:], U3[:], x3ch(0), OP.max)
                V3 = pool.tile([128, F2], F16, tag="sm2", bufs=2)
                nc.vector.tensor_tensor(V3[:], x3ch(1), x3ch(2), OP.min)
                mnc = pool.tile([128, F2], F16, tag="sm8", bufs=2)
                nc.vector.tensor_tensor(mnc[:], V3[:], x3ch(0), OP.min)
                m_r = pool.tile([128, F2], F16, tag="sm4", bufs=2)
                nc.vector.tensor_tensor(m_r[:].bitcast(I16), x3ch(0), U3[:], OP.is_ge)
                m_g = pool.tile([128, F2], F16, tag="sm5", bufs=2)
                nc.vector.tensor_tensor(m_g[:].bitcast(I16), x3ch(1), mxc[:], OP.is_ge)
                cr = pool.tile([128, F2], F16, tag="sm9", bufs=2)
                nc.vector.tensor_tensor(cr[:], mxc[:], mnc[:], OP.subtract)
                d1 = pool.tile([128, F2], F16, tag="sm1", bufs=2)
                nc.vector.tensor_tensor(d1[:], x3ch(1), x3ch(2), OP.subtract)
                d2 = pool.tile([128, F2], F16, tag="sm2", bufs=2)
                nc.vector.tensor_tensor(d2[:], x3ch(2), x3ch(0), OP.subtract)
                lc = pool.tile([128, F2], F32, tag="sm3", bufs=2)
                nc.scalar.activation(lc[:], cr[:], AF.Ln, bias=EPS)
                invc = pool.tile([128, F2], F16, tag="sm6", bufs=2)
                nc.scalar.activation(invc[:], lc[:], AF.Exp, scale=-1.0)
                p1 = pool.tile([128, F2], F16, tag="sm7", bufs=2)
                nc.vector.tensor_tensor(p1[:], d1[:], invc[:], OP.mult)
                p2 = pool.tile([128, F2], F16, tag="sm10", bufs=2)
                nc.vector.tensor_tensor(p2[:], d2[:], invc[:], OP.mult)
                q12 = pool.tile([128, F2], F16, tag="sm1", bufs=2)
                nc.vector.tensor_tensor(q12[:], p1[:], p2[:], OP.add)
                i6g = pool.tile([128, F2], F16, tag="sm2", bufs=2)
                nc.vector.tensor_scalar(i6g[:], p2[:], 2.0, None, OP.add)
                i6 = pool.tile([128, F2], F16, tag="sm11", bufs=2)
                nc.vector.tensor_scalar(i6[:], q12[:], 4.0, -1.0, OP.subtract, OP.mult)
                nc.vector.copy_predicated(i6[:], m_g[:].bitcast(I16), i6g[:])
                nc.vector.copy_predicated(i6[:], m_r[:].bitcast(I16), p1[:])

                # E: tent recon
                t = pool.tile([128, 3 * F2], F16, tag="u", bufs=2)
                for c in range(3):
                    nc.scalar.activation(
                        t[:, c * F2 : (c + 1) * F2], i6[:],
                        AF.Abs, bias=col(i, 6 + c), scale=1.0,
                    )
                wv = pool.tile([128, 3 * F2], F16, tag="wx2", bufs=2)
                nc.scalar.activation(wv[:], t[:], AF.Abs, bias=-3.0)
                qq = pool.tile([128, 3 * F2], F16, tag="sx2", bufs=2)
                nc.vector.tensor_scalar(qq[:], wv[:], 1.0, 2.0, OP.max, OP.min)
                z = pool.tile([128, 3 * F2], F16, tag="y", bufs=2)
                crb = cr[:][:, None, :].broadcast_to([128, 3, F2])
                nc.vector.tensor_tensor(
                    z[:].rearrange("p (c f) -> p c f", c=3),
                    qq[:].rearrange("p (c f) -> p c f", c=3),
                    crb, OP.mult,
                )
                D2 = pool.tile([128, F2], F16, tag="sm10", bufs=2)
                nc.vector.tensor_tensor(D2[:], mnc[:], cr[:], OP.subtract)
                oo = pool.tile([128, 3 * F2], F16, tag="x3", bufs=2)
                D2b = D2[:][:, None, :].broadcast_to([128, 3, F2])
                nc.vector.tensor_tensor(
                    oo[:].rearrange("p (c f) -> p c f", c=3),
                    z[:].rearrange("p (c f) -> p c f", c=3),
                    D2b, OP.add,
                )

                # store chunk: DRAM view [p, c, f-window]
                yv = y_h[i].flatten().rearrange("(c p f) -> p c f", c=3, p=128)
                dma.dma_start(
                    yv[:, :, lo : lo + F2],
                    oo[:].rearrange("p (c f) -> p c f", c=3),
                )

            # software-pipelined emission
            phase1(0)
            phase1(1)
            for k in range(NCH):
                phase2(0, k)
            phase1(2)
            for k in range(NCH):
                phase2(1, k)
            phase1(3)
            for k in range(NCH):
                phase2(2, k)
            for k in range(NCH):
                phase2(3, k)

    nc.finalize()
    return nc


def _get_nc():
    if "nc" not in _CACHE:
        _CACHE["nc"] = _build()
    return _CACHE["nc"]


def kernel(x, brightness_f, contrast_f, saturation_f, hue_f, num_samples=1, **_):
    x16 = np.ascontiguousarray(np.asarray(x, dtype=np.float16))
    bf = np.asarray(brightness_f, np.float32)
    cf = np.asarray(contrast_f, np.float32)
    sf = np.asarray(saturation_f, np.float32)
    hf = np.asarray(hue_f, np.float32)
    w0, w1, w2 = GRAY_W
    fac = np.stack(
        [
            bf, cf, sf,
            (1.0 - sf) * w0, (1.0 - sf) * w1, (1.0 - sf) * w2,
            6.0 * hf, 6.0 * hf - 2.0, 6.0 * hf - 4.0,
            (1.0 - cf) * w0 / np.float32(NPIX),
            (1.0 - cf) * w1 / np.float32(NPIX),
            (1.0 - cf) * w2 / np.float32(NPIX),
        ],
        axis=1,
    ).astype(np.float32)

    nc = _get_nc()
    in_maps = [
        {"x": x16[k * NIMG:(k + 1) * NIMG], "fac": fac[k * NIMG:(k + 1) * NIMG]}
        for k in range(8)
    ]
    res = run_bass_kernel_spmd(nc, in_maps, core_ids=list(range(8)), trace=TRACE)
    if TRACE:
        _CACHE["last"] = res
    out = np.concatenate([res.results[k]["y"] for k in range(8)], axis=0)
    return out.astype(np.float32)


# revision 11
# speedup vs baseline: 1.8323x; 1.2856x over previous
"""ColorRandomizer Trainium2 kernel: brightness/contrast/saturation/hue on 8 cores.

Data-parallel: 4 images per core, f16 I/O (host converts), 2 chunks per image,
software-pipelined front/back halves per chunk for DVE/ACT overlap.

Per image (factors b,c,s,h; W = gray weights):
  A:  x1 = min(b*x, 1); per-channel sums -> m -> d=(1-c)m, e'=min(c+d,1)
  B:  u = relu(c*x1 + d)                       (ACT)
  C:  g = sum_c (1-s)W_c*min(u_c,e');  x3 = clip01(s*min(u,e') + g)
  D:  hue via tent identity, division-free selection:
      p1=(g3-b3)/cr, p2=(b3-r3)/cr, i6 = select(p1 | 2+p2 | 4-p1-p2) by argmax
      (continuity of h6 at clamp ties makes any argmax-consistent pick exact)
  E:  out_c = minc + cr*(clip(| |i6+gam_c| - 3 |, 1, 2) - 1),  gam=(6h,6h-2,6h-4)
"""
import sys

for _p in ("/opt/trn_rl_repo",):
    if _p not in sys.path:
        sys.path.append(_p)

import numpy as np
from concourse import bass, bacc, mybir, tile, bass_isa
from concourse.bass_utils import run_bass_kernel_spmd

F32 = mybir.dt.float32
F16 = mybir.dt.float16
I16 = mybir.dt.int16
OP = mybir.AluOpType
AF = mybir.ActivationFunctionType

NIMG = 4          # images per core
H, W = 480, 640
NPIX = H * W      # 307200
F = NPIX // 128   # 2400 free elems per partition per channel plane
NCH = 2           # chunks per image
F2 = F // NCH     # 1200
GRAY_W = (0.299, 0.587, 0.114)
EPS = 1e-4
NFAC = 12

TRACE = False     # test.py flips this for profiling
_CACHE = {}


def _build():
    nc = bacc.Bacc(None, target_bir_lowering=False)
    x_h = nc.declare_dram_parameter("x", [NIMG, 3, H, W], F16, isOutput=False)
    fac_h = nc.declare_dram_parameter("fac", [NIMG, NFAC], F32, isOutput=False)
    y_h = nc.declare_dram_parameter("y", [NIMG, 3, H, W], F16, isOutput=True)

    dma = nc.sync  # HWDGE

    # activation float biases must exist as const APs
    for v in (EPS, -3.0):
        t = nc.alloc_sbuf_tensor(f"cst-{v}", [128, 1], F32)
        nc.gpsimd.memset(t.ap(), v)
        nc.const_aps.aps[(F32, v)] = t.ap()
    nc.all_engine_barrier()

    with tile.TileContext(nc) as tc:
        with tc.tile_pool(name="p", bufs=1) as pool:
            fac1 = pool.tile([1, NIMG * NFAC], F32)
            dma.dma_start(fac1[:], fac_h[:].flatten()[None, :])
            facb = pool.tile([128, NIMG * NFAC], F32)
            nc.gpsimd.partition_broadcast(facb[:], fac1[:], channels=128)

            def col(i, k):
                return facb[:, i * NFAC + k : i * NFAC + k + 1]

            img = {}   # per-image state
            st = {}    # per-(i,k) cross front/back state

            def phase1(i):
                xin = pool.tile([128, 3 * F], F16, tag="xin", bufs=1)
                xv = x_h[i].flatten().rearrange("(c p f) -> p c f", c=3, p=128)
                dma.dma_start(xin[:].rearrange("p (c f) -> p c f", c=3), xv)
                x1 = pool.tile([128, 3 * F], F16, tag="x1", bufs=2)
                nc.vector.tensor_scalar(x1[:], xin[:], col(i, 0), 1.0, OP.mult, OP.min)
                sums = pool.tile([128, 4], F32, tag="sums", bufs=2)
                for c in range(3):
                    dead = pool.tile([128, F], F16, tag="dead", bufs=1)
                    nc.scalar.activation(
                        dead[:], x1[:, c * F : (c + 1) * F],
                        AF.Identity, bias=0.0, scale=1.0,
                        accum_out=sums[:, c : c + 1],
                    )
                ws = pool.tile([128, 1], F32, tag="ws", bufs=2)
                nc.vector.tensor_scalar(ws[:], sums[:, 0:1], col(i, 9), None, OP.mult)
                ws2 = pool.tile([128, 1], F32, tag="ws2", bufs=2)
                nc.vector.scalar_tensor_tensor(ws2[:], sums[:, 1:2], col(i, 10), ws[:], OP.mult, OP.add)
                ws3 = pool.tile([128, 1], F32, tag="ws3", bufs=2)
                nc.vector.scalar_tensor_tensor(ws3[:], sums[:, 2:3], col(i, 11), ws2[:], OP.mult, OP.add)
                d_t = pool.tile([128, 1], F32, tag="d", bufs=2)
                nc.gpsimd.partition_all_reduce(d_t[:], ws3[:], 128, bass_isa.ReduceOp.add)
                e0 = pool.tile([128, 1], F32, tag="e0", bufs=2)
                nc.vector.scalar_tensor_tensor(e0[:], d_t[:], 1.0, col(i, 1), OP.mult, OP.add)
                e_t = pool.tile([128, 1], F32, tag="e", bufs=2)
                nc.vector.tensor_scalar(e_t[:], e0[:], 1.0, None, OP.min)
                se_t = pool.tile([128, 1], F32, tag="se", bufs=2)
                nc.vector.tensor_tensor(se_t[:], e_t[:], col(i, 2), OP.mult)
                img[i] = (x1, d_t, e_t, se_t)

            def front(i, k):
                x1, d_t, e_t, se_t = img[i]
                lo = k * F2

                # B: u = relu(c*x1 + d)  (one strided-view ACT op)
                u = pool.tile([128, 3 * F2], F16, tag="bigU", bufs=2)
                nc.scalar.activation(
                    u[:].rearrange("p (c f) -> p c f", c=3),
                    x1[:].rearrange("p (c f) -> p c f", c=3)[:, :, lo : lo + F2],
                    AF.Relu, bias=d_t[:], scale=col(i, 1),
                )

                def uch(c):
                    return u[:, c * F2 : (c + 1) * F2]

                # C: gray-blend
                wx2 = pool.tile([128, 3 * F2], F16, tag="bigW", bufs=2)
                for c in range(3):
                    nc.vector.tensor_scalar(
                        wx2[:, c * F2 : (c + 1) * F2], uch(c),
                        e_t[:], col(i, 3 + c), OP.min, OP.mult,
                    )
                ga = pool.tile([128, F2], F16, tag="fs1", bufs=2)
                nc.vector.tensor_tensor(ga[:], wx2[:, 0:F2], wx2[:, F2:2 * F2], OP.add)
                g = pool.tile([128, F2], F16, tag="fs2", bufs=2)
                nc.vector.tensor_tensor(g[:], ga[:], wx2[:, 2 * F2:3 * F2], OP.add)
                sx2 = pool.tile([128, 3 * F2], F16, tag="bigS", bufs=2)
                nc.vector.tensor_scalar(sx2[:], u[:], col(i, 2), se_t[:], OP.mult, OP.min)
                y = pool.tile([128, 3 * F2], F16, tag="bigW", bufs=2)
                gb = g[:][:, None, :].broadcast_to([128, 3, F2])
                nc.vector.tensor_tensor(
                    y[:].rearrange("p (c f) -> p c f", c=3),
                    sx2[:].rearrange("p (c f) -> p c f", c=3),
                    gb, OP.add,
                )
                x3 = pool.tile([128, 3 * F2], F16, tag="bigU", bufs=2)
                nc.vector.tensor_scalar(x3[:], y[:], 0.0, 1.0, OP.max, OP.min)

                def x3ch(c):
                    return x3[:, c * F2 : (c + 1) * F2]

                # D-front: order stats, masks, chroma, diffs, 1/cr
                U3 = pool.tile([128, F2], F16, tag="fs1", bufs=2)
                nc.vector.tensor_tensor(U3[:], x3ch(1), x3ch(2), OP.max)
                mxc = pool.tile([128, F2], F16, tag="fs2", bufs=2)
                nc.vector.tensor_tensor(mxc[:], U3[:], x3ch(0), OP.max)
                V3 = pool.tile([128, F2], F16, tag="fs3", bufs=2)
                nc.vector.tensor_tensor(V3[:], x3ch(1), x3ch(2), OP.min)
                mnc = pool.tile([128, F2], F16, tag="mnc", bufs=2)
                nc.vector.tensor_tensor(mnc[:], V3[:], x3ch(0), OP.min)
                m_r = pool.tile([128, F2], F16, tag="mr", bufs=2)
                nc.vector.tensor_tensor(m_r[:].bitcast(I16), x3ch(0), U3[:], OP.is_ge)
                m_g = pool.tile([128, F2], F16, tag="mg", bufs=2)
                nc.vector.tensor_tensor(m_g[:].bitcast(I16), x3ch(1), mxc[:], OP.is_ge)
                cr = pool.tile([128, F2], F16, tag="cr", bufs=2)
                nc.vector.tensor_tensor(cr[:], mxc[:], mnc[:], OP.subtract)
                d1 = pool.tile([128, F2], F16, tag="d1", bufs=2)
                nc.vector.tensor_tensor(d1[:], x3ch(1), x3ch(2), OP.subtract)
                d2 = pool.tile([128, F2], F16, tag="d2", bufs=2)
                nc.vector.tensor_tensor(d2[:], x3ch(2), x3ch(0), OP.subtract)
                lc = pool.tile([128, F2], F32, tag="lc", bufs=2)
                nc.scalar.activation(lc[:], cr[:], AF.Ln, bias=EPS)
                invc = pool.tile([128, F2], F16, tag="invc", bufs=2)
                nc.scalar.activation(invc[:], lc[:], AF.Exp, scale=-1.0)
                st[(i, k)] = (d1, d2, cr, mnc, invc, m_r, m_g)

            def back(i, k):
                d1, d2, cr, mnc, invc, m_r, m_g = st.pop((i, k))
                lo = k * F2

                p1 = pool.tile([128, F2], F16, tag="p1", bufs=2)
                nc.vector.tensor_tensor(p1[:], d1[:], invc[:], OP.mult)
                p2 = pool.tile([128, F2], F16, tag="bs1", bufs=2)
                nc.vector.tensor_tensor(p2[:], d2[:], invc[:], OP.mult)
                q12 = pool.tile([128, F2], F16, tag="bs2", bufs=2)
                nc.vector.tensor_tensor(q12[:], p1[:], p2[:], OP.add)
                i6g = pool.tile([128, F2], F16, tag="bs3", bufs=2)
                nc.vector.tensor_scalar(i6g[:], p2[:], 2.0, None, OP.add)
                i6 = pool.tile([128, F2], F16, tag="i6", bufs=2)
                nc.vector.tensor_scalar(i6[:], q12[:], 4.0, -1.0, OP.subtract, OP.mult)
                nc.vector.copy_predicated(i6[:], m_g[:].bitcast(I16), i6g[:])
                nc.vector.copy_predicated(i6[:], m_r[:].bitcast(I16), p1[:])

                # E: tent recon
                t = pool.tile([128, 3 * F2], F16, tag="bigA", bufs=2)
                for c in range(3):
                    nc.scalar.activation(
                        t[:, c * F2 : (c + 1) * F2], i6[:],
                        AF.Abs, bias=col(i, 6 + c), scale=1.0,
                    )
                wv = pool.tile([128, 3 * F2], F16, tag="bigB", bufs=2)
                nc.scalar.activation(wv[:], t[:], AF.Abs, bias=-3.0)
                qq = pool.tile([128, 3 * F2], F16, tag="bigA", bufs=2)
                nc.vector.tensor_scalar(qq[:], wv[:], 1.0, 2.0, OP.max, OP.min)
                z = pool.tile([128, 3 * F2], F16, tag="bigB", bufs=2)
                crb = cr[:][:, None, :].broadcast_to([128, 3, F2])
                nc.vector.tensor_tensor(
                    z[:].rearrange("p (c f) -> p c f", c=3),
                    qq[:].rearrange("p (c f) -> p c f", c=3),
                    crb, OP.mult,
                )
                D2 = pool.tile([128, F2], F16, tag="bs2", bufs=2)
                nc.vector.tensor_tensor(D2[:], mnc[:], cr[:], OP.subtract)
                oo = pool.tile([128, 3 * F2], F16, tag="bigA", bufs=2)
                D2b = D2[:][:, None, :].broadcast_to([128, 3, F2])
                nc.vector.tensor_tensor(
                    oo[:].rearrange("p (c f) -> p c f", c=3),
                    z[:].rearrange("p (c f) -> p c f", c=3),
                    D2b, OP.add,
                )
                yv = y_h[i].flatten().rearrange("(c p f) -> p c f", c=3, p=128)
                dma.dma_start(
                    yv[:, :, lo : lo + F2],
                    oo[:].rearrange("p (c f) -> p c f", c=3),
                )

            # software-pipelined emission: front(k+1) before back(k)
            phase1(0)
            phase1(1)
            front(0, 0)
            front(0, 1)
            back(0, 0)
            phase1(2)
            front(1, 0)
            back(0, 1)
            front(1, 1)
            back(1, 0)
            phase1(3)
            front(2, 0)
            back(1, 1)
            front(2, 1)
            back(2, 0)
            front(3, 0)
            back(2, 1)
            front(3, 1)
            back(3, 0)
            back(3, 1)

    nc.finalize()
    return nc


def _get_nc():
    if "nc" not in _CACHE:
        _CACHE["nc"] = _build()
    return _CACHE["nc"]


def kernel(x, brightness_f, contrast_f, saturation_f, hue_f, num_samples=1, **_):
    x16 = np.ascontiguousarray(np.asarray(x, dtype=np.float16))
    bf = np.asarray(brightness_f, np.float32)
    cf = np.asarray(contrast_f, np.float32)
    sf = np.asarray(saturation_f, np.float32)
    hf = np.asarray(hue_f, np.float32)
    w0, w1, w2 = GRAY_W
    fac = np.stack(
        [
            bf, cf, sf,
            (1.0 - sf) * w0, (1.0 - sf) * w1, (1.0 - sf) * w2,
            6.0 * hf, 6.0 * hf - 2.0, 6.0 * hf - 4.0,
            (1.0 - cf) * w0 / np.float32(NPIX),
            (1.0 - cf) * w1 / np.float32(NPIX),
            (1.0 - cf) * w2 / np.float32(NPIX),
        ],
        axis=1,
    ).astype(np.float32)

    nc = _get_nc()
    in_maps = [
        {"x": x16[k * NIMG:(k + 1) * NIMG], "fac": fac[k * NIMG:(k + 1) * NIMG]}
        for k in range(8)
    ]
    res = run_bass_kernel_spmd(nc, in_maps, core_ids=list(range(8)), trace=TRACE)
    if TRACE:
        _CACHE["last"] = res
    out = np.concatenate([res.results[k]["y"] for k in range(8)], axis=0)
    return out.astype(np.float32)
